# revision 26
# baseline (speedup 1.0000x reference)
"""BiLSTM-CRF negative log-likelihood kernel for 8 Trainium2 NeuronCores.

Strategy (data parallel over batch, 64 sequences per core):
  logZ via meet-in-the-middle forward/backward products in normal space.
  Per chain step: one bf16 block-diagonal matmul (E^T / E stationary)
  advancing both half-chains for all 64 sequences, then one DVE multiply
  applying the emission factors exp(feat - MU) (bf16 state/emissions).
  Periodic per-(chain,b) max renorm keeps the range safe; log-scales
  accumulate and are added back at the end.
  Gold-path score via one-hot-mask matmuls (trans gather = trans @ onehot_prev,
  emission gather = masked feats), accumulated in PSUM by ones-matmuls.
  Output: per-core [32,2] per-sequence (logZ - gold); host sums to scalar.
"""

import sys

sys.path.insert(0, "/opt/trn_rl_repo")

import numpy as np
import ml_dtypes

B, S, T = 512, 2048, 32
START_IDX, STOP_IDX = 30, 31
N_CORES = 8
BC = B // N_CORES          # 64 sequences per core
HALF = S // 2              # 1024 chain steps per direction
CHUNK = 32                 # slots per streamed chunk
N_CHUNKS = HALF // CHUNK   # 32
RENORM_EVERY = 256
MU = float(np.log(32.0) + 1.0)   # constant per-step log-baseline removal
SMU = float(S * MU)

BF16 = ml_dtypes.bfloat16


class CFG:
    masked_on_gpsimd = True  # masked-multiply on GPSIMD instead of DVE


def _build_program(cfg=CFG):
    import concourse.bass as bass
    import concourse.tile as tile
    from concourse import bacc, mybir

    dt = mybir.dt
    AF = mybir.ActivationFunctionType
    ALU = mybir.AluOpType
    AX = mybir.AxisListType

    nc = bacc.Bacc("TRN2", target_bir_lowering=False, debug=False,
                   num_devices=N_CORES)

    # ---- DRAM I/O ----
    fmar = nc.dram_tensor("fmar", [64, HALF, BC], dt.bfloat16,
                          kind="ExternalInput").ap()
    maskc = nc.dram_tensor("maskc", [64, HALF, BC], dt.bfloat16,
                           kind="ExternalInput").ap()
    maskp = nc.dram_tensor("maskp", [64, HALF, BC], dt.bfloat16,
                           kind="ExternalInput").ap()
    trans_d = nc.dram_tensor("trans", [T, T], dt.float32,
                             kind="ExternalInput").ap()
    transT_d = nc.dram_tensor("transT", [T, T], dt.float32,
                              kind="ExternalInput").ap()
    blkq_d = nc.dram_tensor("blkq", [64, 64], dt.bfloat16,
                            kind="ExternalInput").ap()
    eye_d = nc.dram_tensor("eye64", [64, 64], dt.bfloat16,
                           kind="ExternalInput").ap()
    tstop_d = nc.dram_tensor("tstop", [T, 1], dt.float32,
                             kind="ExternalInput").ap()
    finit_d = nc.dram_tensor("finit", [T, BC], dt.float32,
                             kind="ExternalInput").ap()
    maskstop_d = nc.dram_tensor("maskstop", [T, BC], dt.bfloat16,
                                kind="ExternalInput").ap()
    maskplast_d = nc.dram_tensor("maskplast", [T, BC], dt.bfloat16,
                                 kind="ExternalInput").ap()
    v0_d = nc.dram_tensor("v0", [T, BC], dt.float32,
                          kind="ExternalInput").ap()
    lossv_d = nc.dram_tensor("lossv", [T, 2], dt.float32,
                             kind="ExternalOutput").ap()

    sdt = dt.bfloat16

    with tile.TileContext(nc) as tc:
        with (
            tc.tile_pool(name="singles", bufs=1) as singles,
            tc.tile_pool(name="state", bufs=4) as state_pool,
            tc.tile_pool(name="stream", bufs=2) as stream,
            tc.tile_pool(name="fpool", bufs=2) as fpool,
            tc.tile_pool(name="mpool", bufs=2) as mpool,
            tc.tile_pool(name="gold", bufs=2) as gold,
            tc.tile_pool(name="rnrm", bufs=2) as rnrm,
            tc.tile_pool(name="tail", bufs=1) as tailp,
            tc.tile_pool(name="ps_chain", bufs=2, space="PSUM") as ps_chain,
            tc.tile_pool(name="ps_q", bufs=2, space="PSUM") as ps_q,
            tc.tile_pool(name="ps_g", bufs=1, space="PSUM") as ps_g,
            tc.tile_pool(name="ps_t", bufs=1, space="PSUM") as ps_tail,
        ):
            # ---------- constants / preamble ----------
            # tmix: rows 0-31 = transT (raw), rows 32-63 = trans (raw)
            tmix = singles.tile([64, T], dt.float32)
            nc.sync.dma_start(tmix[0:32, :], transT_d[:, :])
            nc.sync.dma_start(tmix[32:64, :], trans_d[:, :])
            # gold-side stationaries live on partitions 64-127 (PE tile T10)
            gstat = singles.tile([128, 64], dt.bfloat16)
            nc.sync.dma_start(gstat[64:128, :], blkq_d[:, :])
            geye = singles.tile([128, 64], dt.bfloat16)
            nc.sync.dma_start(geye[64:128, :], eye_d[:, :])
            gones = singles.tile([128, 64], dt.bfloat16)
            nc.vector.memset(gones[64:128, :], 1.0)
            # tS: stop-transition column, both halves
            tS = singles.tile([64, 1], dt.float32)
            nc.sync.dma_start(tS[0:32, :], tstop_d[:, :])
            nc.sync.dma_start(tS[32:64, :], tstop_d[:, :])
            # tF: feats at t=S-1, both halves
            tF = singles.tile([64, BC], dt.float32)
            nc.sync.dma_start(tF[0:32, :], finit_d[:, :])
            nc.sync.dma_start(tF[32:64, :], finit_d[:, :])
            mstop = singles.tile([T, BC], dt.bfloat16)
            nc.sync.dma_start(mstop[:, :], maskstop_d[:, :])
            mplast = singles.tile([T, BC], dt.bfloat16)
            nc.sync.dma_start(mplast[:, :], maskplast_d[:, :])
            mub = singles.tile([64, 1], dt.float32)
            nc.vector.memset(mub[:, :], -MU)

            # chain stationary: block-diag(exp(transT), exp(trans)) in bf16.
            # Exp evaluated with fp32 output (finer act table, avoids the
            # convex-overestimate bias of the 16-bit table), then cast.
            blk32 = singles.tile([64, 64], dt.float32)
            nc.vector.memset(blk32[:, :], 0.0)
            nc.scalar.activation(blk32[0:32, 0:32], tmix[0:32, :], AF.Exp)
            nc.scalar.activation(blk32[32:64, 32:64], tmix[32:64, :], AF.Exp)
            blk = singles.tile([64, 64], dt.bfloat16)
            nc.vector.tensor_copy(blk[:, :], blk32[:, :])
            # final stationary: exp(transT) in top-right block
            blkfin = singles.tile([64, 64], dt.bfloat16)
            nc.vector.memset(blkfin[:, :], 0.0)
            nc.vector.tensor_copy(blkfin[0:32, 32:64], blk32[0:32, 0:32])
            # tail-side transT bf16 (partitions 0-31)
            trTb = singles.tile([T, T], dt.bfloat16)
            nc.vector.tensor_copy(trTb[:, :], tmix[0:32, :])

            ones32f = singles.tile([T, 1], dt.float32)
            nc.vector.memset(ones32f[:, :], 1.0)

            # r = exp(stop transitions), rows 32-63
            r_e = singles.tile([64, 1], dt.float32)
            nc.scalar.activation(r_e[32:64, :], tS[32:64, :], AF.Exp)
            # y0 emission factor exp(feat[S-1] - MU), rows 32-63
            f_last = singles.tile([64, BC], dt.float32)
            nc.scalar.activation(f_last[32:64, :], tF[32:64, :], AF.Exp,
                                 bias=mub[32:64, :])

            # scale-log accumulator [64,2]: (chain, b%32) x (b//32)
            acc = singles.tile([64, 2], dt.float32)
            nc.vector.memset(acc[:, :], 0.0)

            # persistent gold PSUM accumulator (partitions 64-127, tile T10;
            # every row holds the same column sums via the all-ones stationary)
            psg = ps_g.tile([128, 8 * BC], dt.float32)

            # ---------- initial state ----------
            state = state_pool.tile([64, BC], sdt, tag="state")
            v0t = singles.tile([T, BC], dt.float32)
            nc.sync.dma_start(v0t[:, :], v0_d[:, :])
            nc.vector.tensor_copy(state[0:32, :], v0t[:, :])
            nc.vector.tensor_scalar_mul(state[32:64, :], f_last[32:64, :],
                                        r_e[32:64, 0:1])

            # ---------- main loop over chunks ----------
            gold_mm = [0]  # count of accumulating matmuls into psg
            N_ACCUM = N_CHUNKS * 4

            def gold_accum(rhs_ap):
                nc.tensor.matmul(psg[64:128, :], gones[64:128, :], rhs_ap,
                                 start=(gold_mm[0] == 0),
                                 stop=(gold_mm[0] == N_ACCUM - 1),
                                 skip_group_check=True)
                gold_mm[0] += 1

            prev_state = [None, state]  # [state_{i-1}, state_i]

            for ck in range(N_CHUNKS):
                s0 = ck * CHUNK
                raw = stream.tile([64, CHUNK, BC], dt.bfloat16, tag="raw")
                nc.sync.dma_start(raw[:, :, :], fmar[:, s0:s0 + CHUNK, :])
                raw1 = stream.tile([128, CHUNK, BC], dt.bfloat16, tag="raw1")
                nc.sync.dma_start(raw1[64:128, :, :], fmar[:, s0:s0 + CHUNK, :])
                mc = mpool.tile([128, CHUNK, BC], dt.bfloat16, tag="mc")
                nc.sync.dma_start(mc[64:128, :, :], maskc[:, s0:s0 + CHUNK, :])
                mp = mpool.tile([128, CHUNK, BC], dt.bfloat16, tag="mp")
                nc.sync.dma_start(mp[64:128, :, :], maskp[:, s0:s0 + CHUNK, :])

                ftile = fpool.tile([64, CHUNK, BC], dt.float32, tag="f")
                nc.scalar.activation(ftile[:, :, :], raw[:, :, :], AF.Exp,
                                     bias=mub[:, :])

                # ----- gold pipeline on PE tile T10 (partitions 64-127):
                # qq = blkq^T @ mp + I @ raw (two accumulating matmuls), then
                # masked-multiply straight from PSUM on GPSIMD, then the
                # all-ones column-sum matmul accumulates into psg.
                for q in range(4):
                    sl = slice(q * 8, (q + 1) * 8)
                    qp = ps_q.tile([128, 8, BC], dt.float32, tag="qp")
                    nc.tensor.matmul(qp[64:128, :, :], gstat[64:128, :],
                                     mp[64:128, sl, :],
                                     start=True, stop=False,
                                     skip_group_check=True)
                    nc.tensor.matmul(qp[64:128, :, :], geye[64:128, :],
                                     raw1[64:128, sl, :],
                                     start=False, stop=True,
                                     skip_group_check=True)
                    qq = gold.tile([128, 8, BC], dt.bfloat16, tag="qq")
                    nc.scalar.activation(qq[64:128, :, :], qp[64:128, :, :],
                                         AF.Copy)
                    mk = gold.tile([128, 8, BC], dt.bfloat16, tag="mk")
                    nc.gpsimd.tensor_mul(mk[64:128, :, :], qq[64:128, :, :],
                                         mc[64:128, sl, :])
                    gold_accum(mk[64:128, :, :])

                # ----- chain: 32 steps -----
                for j in range(CHUNK):
                    i = s0 + j + 1  # chain step index, 1..1024
                    st_prev = prev_state[1]
                    pu = ps_chain.tile([64, BC], dt.float32, tag="pu")
                    nc.tensor.matmul(pu[:, :], blk[:, :], st_prev[:, :],
                                     start=True, stop=True)
                    st = state_pool.tile([64, BC], sdt, tag="state")
                    nc.vector.tensor_mul(st[:, :], pu[:, :],
                                         ftile[:, j, :])
                    prev_state = [st_prev, st]

                    # periodic renorm (skip the very end; tail handles range)
                    if i % RENORM_EVERY == 0 and i <= HALF - RENORM_EVERY:
                        stf = rnrm.tile([64, 64], dt.float32, tag="stf")
                        nc.vector.tensor_copy(stf[:, :], st[:, :])
                        tst = rnrm.tile([64, 64], dt.float32, tag="tst")
                        nc.vector.transpose(tst[:, :], stf[:, :])
                        m = rnrm.tile([64, 2], dt.float32, tag="m")
                        nc.vector.tensor_reduce(
                            m[:, :],
                            tst[:, :].rearrange("p (c n) -> p c n", n=32),
                            axis=AX.X, op=ALU.max)
                        lg = rnrm.tile([64, 2], dt.float32, tag="lg")
                        nc.scalar.activation(lg[:, :], m[:, :], AF.Ln)
                        nc.vector.tensor_add(acc[:, :], acc[:, :], lg[:, :])
                        rm = rnrm.tile([64, 2], dt.float32, tag="rm")
                        nc.vector.reciprocal(rm[:, :], m[:, :])
                        nc.vector.tensor_scalar_mul(tst[:, 0:32], tst[:, 0:32],
                                                    rm[:, 0:1])
                        nc.vector.tensor_scalar_mul(tst[:, 32:64],
                                                    tst[:, 32:64], rm[:, 1:2])
                        st2f = rnrm.tile([64, 64], dt.float32, tag="st2f")
                        nc.vector.transpose(st2f[:, :], tst[:, :])
                        st2 = state_pool.tile([64, BC], sdt, tag="state")
                        nc.vector.tensor_copy(st2[:, :], st2f[:, :])
                        prev_state = [st_prev, st2]

            # ---------- gold tail: t = S-1 terms ----------
            # stop transition + emission at S-1 + transition (S-2 -> S-1)
            q2 = ps_chain.tile([T, BC], dt.float32, tag="pu")
            nc.tensor.matmul(q2[:, :], trTb[:, :], mplast[:, :],
                             start=True, stop=True)
            g1 = tailp.tile([T, BC], dt.float32)
            nc.vector.tensor_scalar_mul(g1[:, :], mstop[:, :], tS[0:32, 0:1])
            g2 = tailp.tile([T, BC], dt.float32)
            nc.vector.tensor_mul(g2[:, :], mstop[:, :], tF[0:32, :])
            nc.vector.tensor_add(g1[:, :], g1[:, :], g2[:, :])
            g3 = tailp.tile([T, BC], dt.float32)
            nc.vector.tensor_mul(g3[:, :], q2[:, :], mstop[:, :])
            nc.vector.tensor_add(g1[:, :], g1[:, :], g3[:, :])
            q3 = ps_tail.tile([1, BC], dt.float32)
            nc.tensor.matmul(q3[:, :], ones32f[:, :], g1[:, :],
                             start=True, stop=True, skip_group_check=True)
            q3s = tailp.tile([1, BC], dt.float32)
            nc.vector.tensor_copy(q3s[:, :], q3[:, :])

            gold64 = tailp.tile([128, BC], dt.float32)
            nc.vector.tensor_reduce(
                gold64[64:65, :],
                psg[64:65, :].rearrange("p (ls j) -> p j ls", j=BC),
                axis=AX.X, op=ALU.add)

            # ---------- chain tail: dot of the two half-chain states ----------
            st_final = prev_state[1]      # fwd rows hold v_m (after 1024 steps)
            st_bwd = prev_state[0]        # bwd rows hold y_{1023}
            pf = ps_chain.tile([64, BC], dt.float32, tag="pu")
            nc.tensor.matmul(pf[:, :], blkfin[:, :], st_final[:, :],
                             start=True, stop=True)
            prod = tailp.tile([64, BC], dt.float32)
            nc.vector.tensor_mul(prod[32:64, :], pf[32:64, :],
                                 st_bwd[32:64, :])
            tp = tailp.tile([64, BC], dt.float32)
            nc.vector.transpose(tp[32:64, :], prod[32:64, :])
            dotv = tailp.tile([64, 2], dt.float32)
            nc.vector.tensor_reduce(
                dotv[32:64, :],
                tp[32:64, :].rearrange("p (c n) -> p c n", n=32),
                axis=AX.X, op=ALU.add)

            # ---------- combine (all moved to partitions 0-31) ----------
            dot0 = tailp.tile([T, 2], dt.float32)
            nc.sync.dma_start(dot0[:, :], dotv[32:64, :])
            accb0 = tailp.tile([T, 2], dt.float32)
            nc.sync.dma_start(accb0[:, :], acc[32:64, :])
            goldt = tailp.tile([T, 2], dt.float32)
            nc.sync.dma_start(goldt[:, 0:1], gold64[64:65, 0:T])
            nc.sync.dma_start(goldt[:, 1:2], gold64[64:65, T:2 * T])
            tailg = tailp.tile([T, 2], dt.float32)
            nc.sync.dma_start(tailg[:, 0:1], q3s[0:1, 0:T])
            nc.sync.dma_start(tailg[:, 1:2], q3s[0:1, T:2 * T])
            nc.vector.tensor_add(goldt[:, :], goldt[:, :], tailg[:, :])

            lnz = tailp.tile([T, 2], dt.float32)
            nc.scalar.activation(lnz[:, :], dot0[:, :], AF.Ln)
            nc.vector.tensor_add(lnz[:, :], lnz[:, :], acc[0:32, :])
            nc.vector.tensor_add(lnz[:, :], lnz[:, :], accb0[:, :])
            nc.vector.tensor_scalar_add(lnz[:, :], lnz[:, :], SMU)
            nc.vector.tensor_sub(lnz[:, :], lnz[:, :], goldt[:, :])
            nc.sync.dma_start(lossv_d[:, :], lnz[:, :])

    nc.compile()
    return nc


def _marshal(feats, transitions, tags):
    feats = np.asarray(feats, dtype=np.float32)
    transitions = np.asarray(transitions, dtype=np.float32)
    tags = np.asarray(tags)
    eye = np.arange(T, dtype=tags.dtype)

    trans = np.ascontiguousarray(transitions)
    transT = np.ascontiguousarray(transitions.T)
    tstop = np.ascontiguousarray(transitions[STOP_IDX, :].reshape(T, 1))
    blkq = np.zeros((64, 64), dtype=BF16)
    blkq[0:T, 0:T] = transT.astype(BF16)
    blkq[T:2 * T, T:2 * T] = transT.astype(BF16)
    eye64 = np.eye(64, dtype=np.float32).astype(BF16)

    in_maps = []
    for c in range(N_CORES):
        b0, b1 = c * BC, (c + 1) * BC
        f = feats[b0:b1]          # [64, 2048, 32]
        tg = tags[b0:b1]          # [64, 2048]

        fmar = np.zeros((64, HALF, BC), dtype=BF16)
        fmar[0:32] = f[:, 0:HALF, :].transpose(2, 1, 0).astype(BF16)
        # bwd slot s holds feat t=2046-s (slot HALF-1 is zero padding)
        fmar[32:64, 0:HALF - 1] = (
            f[:, HALF:S - 1, :][:, ::-1, :].transpose(2, 1, 0).astype(BF16))

        # one-hot masks; bwd rows cover t=2046-s to match fmar
        mc = np.zeros((64, HALF, BC), dtype=BF16)
        mp = np.zeros((64, HALF, BC), dtype=BF16)
        oh_f = (tg[:, 0:HALF, None] == eye).transpose(2, 1, 0)
        mc[0:32] = oh_f.astype(BF16)
        oh_b = (tg[:, HALF:S - 1, None] == eye)[:, ::-1, :].transpose(2, 1, 0)
        mc[32:64, 0:HALF - 1] = oh_b.astype(BF16)
        tprev = np.concatenate(
            [np.full((BC, 1), START_IDX, dtype=tg.dtype), tg[:, :-1]], axis=1)
        ohp_f = (tprev[:, 0:HALF, None] == eye).transpose(2, 1, 0)
        mp[0:32] = ohp_f.astype(BF16)
        ohp_b = (tprev[:, HALF:S - 1, None] == eye)[:, ::-1, :].transpose(2, 1, 0)
        mp[32:64, 0:HALF - 1] = ohp_b.astype(BF16)

        finit = np.ascontiguousarray(f[:, S - 1, :].T)          # [32, 64]
        maskstop = np.ascontiguousarray(
            (tg[:, S - 1, None] == eye).T.astype(BF16))
        maskplast = np.ascontiguousarray(
            (tg[:, S - 2, None] == eye).T.astype(BF16))

        v0 = np.zeros((T, BC), dtype=np.float32)
        v0[START_IDX, :] = 1.0
        in_maps.append({
            "v0": v0,
            "fmar": fmar, "maskc": mc, "maskp": mp,
            "trans": trans, "transT": transT, "tstop": tstop,
            "blkq": blkq, "eye64": eye64,
            "finit": finit, "maskstop": maskstop, "maskplast": maskplast,
        })
    return in_maps


_PROGRAM = [None]
TRACE = False
TRACE_KW = {}
LAST_EXEC_NS = None
LAST_RESULT = [None]


def kernel(feats, transitions, tags):
    global LAST_EXEC_NS
    from concourse.bass_utils import run_bass_kernel_spmd

    if _PROGRAM[0] is None:
        _PROGRAM[0] = _build_program()
    nc = _PROGRAM[0]
    in_maps = _marshal(feats, transitions, tags)
    res = run_bass_kernel_spmd(nc, in_maps, list(range(N_CORES)),
                               trace=TRACE, **TRACE_KW)
    LAST_EXEC_NS = res.exec_time_ns
    LAST_RESULT[0] = res
    total = np.float32(0.0)
    for c in range(N_CORES):
        lv = res.results[c]["lossv"]  # [32, 2]: b = 32*col + row
        total = np.float32(total + np.sum(lv, dtype=np.float32))
    return np.asarray(total, dtype=np.float32)



# revision 41
# speedup vs baseline: 1.6259x; 1.6259x over previous
"""BiLSTM-CRF negative log-likelihood kernel for 8 Trainium2 NeuronCores.

Strategy (data parallel over batch, 64 sequences per core):
  logZ via SEGMENTED normal-space chains: the sequence is cut into K=8
  segments of L=256 steps. Products of positive matrices contract to
  rank-1 (Birkhoff), so each interior segment's transfer matrix M_k is
  numerically rank-1 and is represented by its action on a uniform probe:
  f_k = M_k u (fwd chain), g_k^T = u^T M_k (bwd chain), c_k = u^T f_k.
  logZ = ln(g_1^T a_0) + sum_k ln(g_{k+1}^T f_k) - sum ln c_k + S*MU,
  with exact end chains (a_0 from the one-hot START vector, b from the
  stop-transition vector). All 7 (fwd, bwd) chain pairs advance together:
  one bf16 64x64-stationary matmul over 448 columns per round, split into
  two half-width streams so the DVE emission-multiply of one half overlaps
  the PE matmul of the other. 256 serial rounds instead of 1024.

  Gold-path score on PE tile T10 (partitions 64-127), fully overlapped
  with the chain on tile T0: qq = blkq^T @ mask_prev + I @ feats via two
  accumulating matmuls, ScalarE Copy evacuation, GPSIMD mask-multiply,
  and an all-ones column-sum matmul accumulating into PSUM.
  Output: per-core [32,2] per-sequence (logZ - gold); host sums to scalar.
"""

import sys

sys.path.insert(0, "/opt/trn_rl_repo")

import numpy as np
import ml_dtypes

B, S, T = 512, 2048, 32
START_IDX, STOP_IDX = 30, 31
N_CORES = 8
BC = B // N_CORES          # 64 sequences per core
K_SEG = 8                  # segments
L_SEG = S // K_SEG         # 256 rounds (serial depth)
NPAIR = K_SEG - 1          # 7 chain pairs
CCOLS = NPAIR * BC         # 448 chain columns
CHALF = CCOLS // 2         # 224 per half-stream
CH = 8                     # rounds per streamed chunk
N_CCHUNK = L_SEG // CH     # 32 chain chunks
HALF = S // 2              # gold-side slot count (1024)
GCHUNK = 32                # gold slots per chunk
N_GCHUNK = HALF // GCHUNK  # 32
MU = float(np.log(32.0) + 1.0)   # per-step log-baseline removal
DRIFT = 0.105                    # measured mean log-drift per chain step
MU_EFF = MU - DRIFT              # keeps typical chain magnitude ~1
SMU = float(S * MU_EFF)

BF16 = ml_dtypes.bfloat16


def _build_program():
    import concourse.bass as bass
    import concourse.tile as tile
    from concourse import bacc, mybir

    dt = mybir.dt
    AF = mybir.ActivationFunctionType
    ALU = mybir.AluOpType
    AX = mybir.AxisListType

    nc = bacc.Bacc("TRN2", target_bir_lowering=False, debug=False,
                   num_devices=N_CORES)

    # ---- DRAM I/O ----
    fchain = nc.dram_tensor("fchain", [64, L_SEG, CCOLS], dt.bfloat16,
                            kind="ExternalInput").ap()
    fmar = nc.dram_tensor("fmar", [64, HALF, BC], dt.bfloat16,
                          kind="ExternalInput").ap()
    maskc = nc.dram_tensor("maskc", [64, HALF, BC], dt.bfloat16,
                           kind="ExternalInput").ap()
    maskp = nc.dram_tensor("maskp", [64, HALF, BC], dt.bfloat16,
                           kind="ExternalInput").ap()
    trans_d = nc.dram_tensor("trans", [T, T], dt.float32,
                             kind="ExternalInput").ap()
    transT_d = nc.dram_tensor("transT", [T, T], dt.float32,
                              kind="ExternalInput").ap()
    blkq_d = nc.dram_tensor("blkq", [64, 64], dt.bfloat16,
                            kind="ExternalInput").ap()
    eye_d = nc.dram_tensor("eye64", [64, 64], dt.bfloat16,
                           kind="ExternalInput").ap()
    tstop_d = nc.dram_tensor("tstop", [T, 1], dt.float32,
                             kind="ExternalInput").ap()
    finit_d = nc.dram_tensor("finit", [T, BC], dt.float32,
                             kind="ExternalInput").ap()
    maskstop_d = nc.dram_tensor("maskstop", [T, BC], dt.bfloat16,
                                kind="ExternalInput").ap()
    maskplast_d = nc.dram_tensor("maskplast", [T, BC], dt.bfloat16,
                                 kind="ExternalInput").ap()
    v0_d = nc.dram_tensor("v0", [T, BC], dt.bfloat16,
                          kind="ExternalInput").ap()
    lossv_d = nc.dram_tensor("lossv", [T, 2], dt.float32,
                             kind="ExternalOutput").ap()
    dbg_d = nc.dram_tensor("dbg", [1, 2 * CCOLS], dt.float32,
                           kind="ExternalOutput").ap()
    dbgst_d = nc.dram_tensor("dbgst", [64, 2 * CHALF], dt.bfloat16,
                             kind="ExternalOutput").ap()

    with tile.TileContext(nc) as tc:
        with (
            tc.tile_pool(name="singles", bufs=1) as singles,
            tc.tile_pool(name="sta", bufs=4) as sta_pool,
            tc.tile_pool(name="stb", bufs=4) as stb_pool,
            tc.tile_pool(name="stream", bufs=2) as stream,
            tc.tile_pool(name="fpool", bufs=2) as fpool,
            tc.tile_pool(name="mpool", bufs=2) as mpool,
            tc.tile_pool(name="gold", bufs=2) as gold,
            tc.tile_pool(name="tail", bufs=1) as tailp,
            tc.tile_pool(name="ps_a", bufs=2, space="PSUM") as ps_a,
            tc.tile_pool(name="ps_b", bufs=2, space="PSUM") as ps_b,
            tc.tile_pool(name="ps_q", bufs=2, space="PSUM") as ps_q,
            tc.tile_pool(name="ps_g", bufs=1, space="PSUM") as ps_g,
        ):
            # ---------- constants / preamble ----------
            # tmix: rows 0-31 = transT (raw), rows 32-63 = trans (raw)
            tmix = singles.tile([64, T], dt.float32)
            nc.sync.dma_start(tmix[0:32, :], transT_d[:, :])
            nc.sync.dma_start(tmix[32:64, :], trans_d[:, :])
            # gold-side stationaries on partitions 64-127 (PE tile T10)
            gstat = singles.tile([128, 64], dt.bfloat16)
            nc.sync.dma_start(gstat[64:128, :], blkq_d[:, :])
            geye = singles.tile([128, 64], dt.bfloat16)
            nc.sync.dma_start(geye[64:128, :], eye_d[:, :])
            gones = singles.tile([128, 64], dt.bfloat16)
            nc.vector.memset(gones[64:128, :], 1.0)
            # tail-side transT bf16 (partitions 0-31)
            trTb = singles.tile([T, T], dt.bfloat16)
            nc.vector.tensor_copy(trTb[:, :], tmix[0:32, :])
            # tS: stop-transition column (both partition halves),
            # tF: feats at t=S-1 (gold tail rows 0-31, chain bwd init 32-63)
            tS = singles.tile([64, 1], dt.float32)
            nc.sync.dma_start(tS[0:32, :], tstop_d[:, :])
            nc.sync.dma_start(tS[32:64, :], tstop_d[:, :])
            tF = singles.tile([64, BC], dt.float32)
            nc.sync.dma_start(tF[0:32, :], finit_d[:, :])
            nc.sync.dma_start(tF[32:64, :], finit_d[:, :])
            mstop = singles.tile([T, BC], dt.bfloat16)
            nc.sync.dma_start(mstop[:, :], maskstop_d[:, :])
            mplast = singles.tile([T, BC], dt.bfloat16)
            nc.sync.dma_start(mplast[:, :], maskplast_d[:, :])
            mub = singles.tile([64, 1], dt.float32)
            nc.vector.memset(mub[:, :], -MU_EFF)

            # chain stationary: block-diag(exp(transT), exp(trans)) bf16.
            # Exp in fp32, then cast.
            blk32 = singles.tile([64, 64], dt.float32)
            nc.vector.memset(blk32[:, :], 0.0)
            nc.scalar.activation(blk32[0:32, 0:32], tmix[0:32, :], AF.Exp)
            nc.scalar.activation(blk32[32:64, 32:64], tmix[32:64, :], AF.Exp)
            blk = singles.tile([64, 64], dt.bfloat16)
            nc.vector.tensor_copy(blk[:, :], blk32[:, :])

            ones32f = singles.tile([T, 1], dt.float32)
            nc.vector.memset(ones32f[:, :], 1.0)
            ones32b = singles.tile([T, 1], dt.bfloat16)
            nc.vector.memset(ones32b[:, :], 1.0)
            # r = exp(stop transitions) on rows 32-63 for the true-bwd init
            r_e = singles.tile([64, 1], dt.float32)
            nc.scalar.activation(r_e[32:64, :], tS[32:64, :], AF.Exp)
            # f_last = exp(feat[S-1] - MU) on rows 32-63
            f_last = singles.tile([64, BC], dt.float32)
            nc.scalar.activation(f_last[32:64, :], tF[32:64, :], AF.Exp,
                                 bias=mub[32:64, :])

            # persistent gold PSUM accumulator (partitions 64-127, T10)
            psg = ps_g.tile([128, 8 * BC], dt.float32)

            # ---------- initial chain state ----------
            # pair p cols [64p, 64p+64): rows 0-31 fwd(seg p), 32-63 bwd(seg p+1)
            stA = sta_pool.tile([64, CHALF], dt.bfloat16, tag="stA")
            nc.vector.memset(stA[:, :], 1.0)
            nc.sync.dma_start(stA[0:32, 0:BC], v0_d[:, :])  # seg-0 fwd = v0
            stB = stb_pool.tile([64, CHALF], dt.bfloat16, tag="stB")
            nc.vector.memset(stB[:, :], 1.0)
            # pair 6 (global cols 384-447 = B-cols 160-223):
            # end-bwd init z = e_{S-1} * r
            nc.vector.tensor_scalar_mul(stB[32:64, 160:224],
                                        f_last[32:64, :], r_e[32:64, 0:1])

            # ---------- gold accumulation helper ----------
            gold_mm = [0]
            N_ACCUM = N_GCHUNK * 4

            def gold_accum(rhs_ap):
                nc.tensor.matmul(psg[64:128, :], gones[64:128, :], rhs_ap,
                                 start=(gold_mm[0] == 0),
                                 stop=(gold_mm[0] == N_ACCUM - 1),
                                 skip_group_check=True)
                gold_mm[0] += 1

            # ---------- gold chunks (tile T10, partitions 64-127) ----------
            for ck in range(N_GCHUNK):
                s0 = ck * GCHUNK
                raw1 = stream.tile([128, GCHUNK, BC], dt.bfloat16, tag="raw1")
                nc.sync.dma_start(raw1[64:128, :, :], fmar[:, s0:s0 + GCHUNK, :])
                mc = mpool.tile([128, GCHUNK, BC], dt.bfloat16, tag="mc")
                nc.sync.dma_start(mc[64:128, :, :], maskc[:, s0:s0 + GCHUNK, :])
                mp = mpool.tile([128, GCHUNK, BC], dt.bfloat16, tag="mp")
                nc.sync.dma_start(mp[64:128, :, :], maskp[:, s0:s0 + GCHUNK, :])

                for q in range(4):
                    sl = slice(q * 8, (q + 1) * 8)
                    qp = ps_q.tile([128, 8, BC], dt.float32, tag="qp")
                    nc.tensor.matmul(qp[64:128, :, :], gstat[64:128, :],
                                     mp[64:128, sl, :],
                                     start=True, stop=False,
                                     skip_group_check=True)
                    nc.tensor.matmul(qp[64:128, :, :], geye[64:128, :],
                                     raw1[64:128, sl, :],
                                     start=False, stop=True,
                                     skip_group_check=True)
                    qq = gold.tile([128, 8, BC], dt.bfloat16, tag="qq")
                    nc.scalar.activation(qq[64:128, :, :], qp[64:128, :, :],
                                         AF.Copy)
                    mk = gold.tile([128, 8, BC], dt.bfloat16, tag="mk")
                    nc.gpsimd.tensor_mul(mk[64:128, :, :], qq[64:128, :, :],
                                         mc[64:128, sl, :])
                    gold_accum(mk[64:128, :, :])

            # ---------- chain rounds (tile T0, partitions 0-63) ----------
            prevA, prevB = stA, stB
            for ck in range(N_CCHUNK):
                r0 = ck * CH
                raw0 = stream.tile([64, CH, CCOLS], dt.bfloat16, tag="raw0")
                nc.sync.dma_start(raw0[:, :, :], fchain[:, r0:r0 + CH, :])
                ftile = fpool.tile([64, CH, CCOLS], dt.float32, tag="f")
                nc.scalar.activation(ftile[:, :, :], raw0[:, :, :], AF.Exp,
                                     bias=mub[:, :])
                for j in range(CH):
                    puA = ps_a.tile([64, CHALF], dt.float32, tag="puA")
                    nc.tensor.matmul(puA[:, :], blk[:, :], prevA[:, :],
                                     start=True, stop=True)
                    nA = sta_pool.tile([64, CHALF], dt.bfloat16, tag="stA")
                    nc.vector.tensor_mul(nA[:, :], puA[:, :],
                                         ftile[:, j, 0:CHALF])
                    puB = ps_b.tile([64, CHALF], dt.float32, tag="puB")
                    nc.tensor.matmul(puB[:, :], blk[:, :], prevB[:, :],
                                     start=True, stop=True)
                    nB = stb_pool.tile([64, CHALF], dt.bfloat16, tag="stB")
                    nc.vector.tensor_mul(nB[:, :], puB[:, :],
                                         ftile[:, j, CHALF:CCOLS])
                    prevA, prevB = nA, nB

            # ---------- chain tail: junction dots + c-corrections ----------
            # erow stationary: rows 32-63 hold rowsums of E (= E @ 1),
            # computed via a blk matmul against an all-ones vector.
            onesv = singles.tile([64, 1], dt.bfloat16)
            nc.vector.memset(onesv[:, :], 1.0)
            erow_ps = ps_b.tile([64, 1], dt.float32, tag="puB")
            nc.tensor.matmul(erow_ps[:, :], blk[:, :], onesv[:, :],
                             start=True, stop=True)
            erow_sb = tailp.tile([64, 1], dt.bfloat16)
            nc.vector.tensor_copy(erow_sb[0:32, :], erow_ps[0:32, :])
            erowst = tailp.tile([64, 1], dt.bfloat16)
            nc.vector.memset(erowst[:, :], 0.0)
            nc.sync.dma_start(erowst[32:64, :], erow_sb[0:32, :])

            # move the bwd halves down to partitions 0-31 (lane-aligned muls)
            bwA = tailp.tile([T, CHALF], dt.bfloat16)
            nc.sync.dma_start(bwA[:, :], prevA[32:64, :])
            bwB = tailp.tile([T, CHALF], dt.bfloat16)
            nc.sync.dma_start(bwB[:, :], prevB[32:64, :])
            # Ef = blk^T @ st_final: rows 0-31 hold E @ f
            efA = ps_a.tile([64, CHALF], dt.float32, tag="puA")
            nc.tensor.matmul(efA[:, :], blk[:, :], prevA[:, :],
                             start=True, stop=True)
            efB = ps_b.tile([64, CHALF], dt.float32, tag="puB")
            nc.tensor.matmul(efB[:, :], blk[:, :], prevB[:, :],
                             start=True, stop=True)
            # junction products: pairs 0-5 use (E f) * g ; pair 6 (end,
            # B-cols 160-223) uses f * beta directly
            prodA = tailp.tile([T, CHALF], dt.bfloat16)
            nc.vector.tensor_mul(prodA[:, :], efA[0:32, :], bwA[:, :])
            prodB = tailp.tile([T, CHALF], dt.bfloat16)
            nc.vector.tensor_mul(prodB[:, 0:160], efB[0:32, 0:160],
                                 bwB[:, 0:160])
            nc.vector.tensor_mul(prodB[:, 160:224], prevB[0:32, 160:224],
                                 bwB[:, 160:224])
            dotA = ps_a.tile([1, CHALF], dt.float32, tag="puA")
            nc.tensor.matmul(dotA[:, :], ones32b[:, :], prodA[:, :],
                             start=True, stop=True)
            dotB = ps_b.tile([1, CHALF], dt.float32, tag="puB")
            nc.tensor.matmul(dotB[:, :], ones32b[:, :], prodB[:, :],
                             start=True, stop=True)
            # c_k = g_k^T (E 1): weighted column sums of the bwd finals
            csA = ps_a.tile([1, CHALF], dt.float32, tag="puA")
            nc.tensor.matmul(csA[:, :], erowst[:, :], prevA[:, :],
                             start=True, stop=True)
            csB = ps_b.tile([1, CHALF], dt.float32, tag="puB")
            nc.tensor.matmul(csB[:, :], erowst[:, :], prevB[:, :],
                             start=True, stop=True)

            dots = tailp.tile([1, CCOLS], dt.float32)
            nc.vector.tensor_copy(dots[:, 0:CHALF], dotA[:, :])
            nc.vector.tensor_copy(dots[:, CHALF:CCOLS], dotB[:, :])
            csum = tailp.tile([1, CCOLS], dt.float32)
            nc.vector.tensor_copy(csum[:, 0:CHALF], csA[:, :])
            nc.vector.tensor_copy(csum[:, CHALF:CCOLS], csB[:, :])

            lnd = tailp.tile([1, CCOLS], dt.float32)
            nc.scalar.activation(lnd[:, :], dots[:, :], AF.Ln)
            lnc = tailp.tile([1, CCOLS], dt.float32)
            nc.scalar.activation(lnc[:, :], csum[:, :], AF.Ln)

            sumd = tailp.tile([1, BC], dt.float32)
            nc.vector.tensor_reduce(
                sumd[:, :],
                lnd[:, :].rearrange("p (k b) -> p b k", k=NPAIR),
                axis=AX.X, op=ALU.add)
            # c-corrections: interior segments 1..6 live at pair cols 0..5
            sumc = tailp.tile([1, BC], dt.float32)
            nc.vector.tensor_reduce(
                sumc[:, :],
                lnc[:, 0:(NPAIR - 1) * BC].rearrange("p (k b) -> p b k",
                                                     k=NPAIR - 1),
                axis=AX.X, op=ALU.add)
            lnz64 = tailp.tile([1, BC], dt.float32)
            nc.vector.tensor_sub(lnz64[:, :], sumd[:, :], sumc[:, :])
            nc.vector.tensor_scalar_add(lnz64[:, :], lnz64[:, :], SMU)

            # ---------- gold tail: t = S-1 terms ----------
            q2 = ps_q.tile([T, BC], dt.float32, tag="qp")
            nc.tensor.matmul(q2[:, :], trTb[:, :], mplast[:, :],
                             start=True, stop=True)
            g1 = tailp.tile([T, BC], dt.float32)
            nc.vector.tensor_scalar_mul(g1[:, :], mstop[:, :], tS[0:32, 0:1])
            g2 = tailp.tile([T, BC], dt.float32)
            nc.vector.tensor_mul(g2[:, :], mstop[:, :], tF[0:32, :])
            nc.vector.tensor_add(g1[:, :], g1[:, :], g2[:, :])
            g3 = tailp.tile([T, BC], dt.float32)
            nc.vector.tensor_mul(g3[:, :], q2[:, :], mstop[:, :])
            nc.vector.tensor_add(g1[:, :], g1[:, :], g3[:, :])
            q3 = ps_q.tile([1, BC], dt.float32, tag="qp")
            nc.tensor.matmul(q3[:, :], ones32f[:, :], g1[:, :],
                             start=True, stop=True, skip_group_check=True)
            q3s = tailp.tile([1, BC], dt.float32)
            nc.vector.tensor_copy(q3s[:, :], q3[:, :])

            gold64 = tailp.tile([128, BC], dt.float32)
            nc.vector.tensor_reduce(
                gold64[64:65, :],
                psg[64:65, :].rearrange("p (ls j) -> p j ls", j=BC),
                axis=AX.X, op=ALU.add)

            # ---------- combine to [32, 2] ----------
            lnzt = tailp.tile([T, 2], dt.float32)
            nc.sync.dma_start(lnzt[:, 0:1], lnz64[0:1, 0:T])
            nc.sync.dma_start(lnzt[:, 1:2], lnz64[0:1, T:2 * T])
            goldt = tailp.tile([T, 2], dt.float32)
            nc.sync.dma_start(goldt[:, 0:1], gold64[64:65, 0:T])
            nc.sync.dma_start(goldt[:, 1:2], gold64[64:65, T:2 * T])
            tailg = tailp.tile([T, 2], dt.float32)
            nc.sync.dma_start(tailg[:, 0:1], q3s[0:1, 0:T])
            nc.sync.dma_start(tailg[:, 1:2], q3s[0:1, T:2 * T])
            nc.vector.tensor_add(goldt[:, :], goldt[:, :], tailg[:, :])

            lossv = tailp.tile([T, 2], dt.float32)
            nc.vector.tensor_sub(lossv[:, :], lnzt[:, :], goldt[:, :])
            nc.sync.dma_start(lossv_d[:, :], lossv[:, :])

            # debug dumps
            nc.sync.dma_start(dbg_d[:, 0:CCOLS], dots[:, :])
            nc.sync.dma_start(dbg_d[:, CCOLS:2 * CCOLS], csum[:, :])
            nc.sync.dma_start(dbgst_d[:, 0:CHALF], prevA[:, :])
            nc.sync.dma_start(dbgst_d[:, CHALF:2 * CHALF], prevB[:, :])

    nc.compile()
    return nc


def _marshal(feats, transitions, tags):
    feats = np.asarray(feats, dtype=np.float32)
    transitions = np.asarray(transitions, dtype=np.float32)
    tags = np.asarray(tags)
    eye = np.arange(T, dtype=tags.dtype)

    trans = np.ascontiguousarray(transitions)
    transT = np.ascontiguousarray(transitions.T)
    tstop = np.ascontiguousarray(transitions[STOP_IDX, :].reshape(T, 1))
    blkq = np.zeros((64, 64), dtype=BF16)
    blkq[0:T, 0:T] = transT.astype(BF16)
    blkq[T:2 * T, T:2 * T] = transT.astype(BF16)
    eye64 = np.eye(64, dtype=np.float32).astype(BF16)

    in_maps = []
    for c in range(N_CORES):
        b0, b1 = c * BC, (c + 1) * BC
        f = feats[b0:b1]          # [64, 2048, 32]
        tg = tags[b0:b1]          # [64, 2048]

        # chain emissions, segmented layout [64, L, (pair, b)]
        fchain = np.zeros((64, L_SEG, CCOLS), dtype=BF16)
        for p in range(NPAIR):
            fw = f[:, p * L_SEG:(p + 1) * L_SEG, :]          # [b, L, T]
            fchain[0:32, :, p * BC:(p + 1) * BC] = (
                fw.transpose(2, 1, 0).astype(BF16))
            if p < NPAIR - 1:
                # interior bwd: round r processes t = (p+2)L-1-r
                bw = f[:, (p + 1) * L_SEG:(p + 2) * L_SEG, :][:, ::-1, :]
                fchain[32:64, :, p * BC:(p + 1) * BC] = (
                    bw.transpose(2, 1, 0).astype(BF16))
            else:
                # end bwd: init absorbs e_{S-1}; rounds r=0..L-2 process
                # t = S-2-r; round L-1 is a pad with emission exactly 1
                bw = f[:, (p + 1) * L_SEG:S - 1, :][:, ::-1, :]  # [b, L-1, T]
                fchain[32:64, 0:L_SEG - 1, p * BC:(p + 1) * BC] = (
                    bw.transpose(2, 1, 0).astype(BF16))
                fchain[32:64, L_SEG - 1, p * BC:(p + 1) * BC] = BF16(MU_EFF)

        # gold layout (fwd half + reversed bwd half, slot HALF-1 padding)
        fmar = np.zeros((64, HALF, BC), dtype=BF16)
        fmar[0:32] = f[:, 0:HALF, :].transpose(2, 1, 0).astype(BF16)
        fmar[32:64, 0:HALF - 1] = (
            f[:, HALF:S - 1, :][:, ::-1, :].transpose(2, 1, 0).astype(BF16))

        mc = np.zeros((64, HALF, BC), dtype=BF16)
        mp = np.zeros((64, HALF, BC), dtype=BF16)
        oh_f = (tg[:, 0:HALF, None] == eye).transpose(2, 1, 0)
        mc[0:32] = oh_f.astype(BF16)
        oh_b = (tg[:, HALF:S - 1, None] == eye)[:, ::-1, :].transpose(2, 1, 0)
        mc[32:64, 0:HALF - 1] = oh_b.astype(BF16)
        tprev = np.concatenate(
            [np.full((BC, 1), START_IDX, dtype=tg.dtype), tg[:, :-1]], axis=1)
        ohp_f = (tprev[:, 0:HALF, None] == eye).transpose(2, 1, 0)
        mp[0:32] = ohp_f.astype(BF16)
        ohp_b = (tprev[:, HALF:S - 1, None] == eye)[:, ::-1, :].transpose(2, 1, 0)
        mp[32:64, 0:HALF - 1] = ohp_b.astype(BF16)

        finit = np.ascontiguousarray(f[:, S - 1, :].T)          # [32, 64]
        maskstop = np.ascontiguousarray(
            (tg[:, S - 1, None] == eye).T.astype(BF16))
        maskplast = np.ascontiguousarray(
            (tg[:, S - 2, None] == eye).T.astype(BF16))

        v0 = np.zeros((T, BC), dtype=BF16)
        v0[START_IDX, :] = 1.0
        in_maps.append({
            "v0": v0, "fchain": fchain,
            "fmar": fmar, "maskc": mc, "maskp": mp,
            "trans": trans, "transT": transT, "tstop": tstop,
            "blkq": blkq, "eye64": eye64,
            "finit": finit, "maskstop": maskstop, "maskplast": maskplast,
        })
    return in_maps


_PROGRAM = [None]
TRACE = False
TRACE_KW = {}
LAST_EXEC_NS = None
LAST_RESULT = [None]


def kernel(feats, transitions, tags):
    global LAST_EXEC_NS
    from concourse.bass_utils import run_bass_kernel_spmd

    if _PROGRAM[0] is None:
        _PROGRAM[0] = _build_program()
    nc = _PROGRAM[0]
    in_maps = _marshal(feats, transitions, tags)
    res = run_bass_kernel_spmd(nc, in_maps, list(range(N_CORES)),
                               trace=TRACE, **TRACE_KW)
    LAST_EXEC_NS = res.exec_time_ns
    LAST_RESULT[0] = res
    total = np.float32(0.0)
    for c in range(N_CORES):
        lv = res.results[c]["lossv"]  # [32, 2]: b = 32*col + row
        total = np.float32(total + np.sum(lv, dtype=np.float32))
    return np.asarray(total, dtype=np.float32)
